# revision 4
# baseline (speedup 1.0000x reference)
"""Trainium2 Bass kernel for modulated conv1d (StyleGAN-style Conv1DMod).

Reference computation (per batch sample b):
  wm[k,c,f]  = kern[k,c,f] * coef * (style[b,c] + 1)        (modulate)
  denom[f]   = rsqrt(sum_{k,c} wm[k,c,f]^2)                 (demodulate)
  out[b,f,w] = denom[f] * sum_{k,c} wm[k,c,f] * feat[b,c,w+k-1]   (SAME conv)

Sharding: data-parallel over batch B=8 -> one sample per NeuronCore.
Demodulation is a per-(b,f) linear scale, so it is applied to the conv
*output* tiles (whose partition dim is f) instead of rescaling weights.

v3 structure (from the v1 trace: first MM at 14.6us, PE cold till 23.7us):
  - the conv runs in bf16 (weights and features; fp32 PSUM accumulate).
    bf16 keeps the PE at 1 col/cycle like fp32r but has no
    "producer must round" verifier constraint and enables FWL weight loads
  - chunk 0 lands fp32 via the two HWDGE queues (ct0 on sync, ct1 on
    scalar) in 4 pieces each and is converted to bf16 by the scalar (ct1)
    and vector (ct0) engines; chunks 1-3 load via SWDGE (gpsimd)
    cast-DMA, which converts fp32->bf16 inside the DMA engines
  - a block of dummy bf16 matmuls at the head keeps the PE busy during
    the initial DMA wait so the HAM clock-gate opens (K=8/8) before the
    real matmul stream starts
  - weight-major matmul order: one (ct,k) weight load feeds the 4
    accumulating matmuls of a chunk (LDWEIGHTS 4x amortized)
  - conv output is demodulated into bf16 staging tiles and stored as
    bf16 (halves store traffic + kernel tail); host upcasts to fp32
"""

import numpy as np

import concourse.bass as bass
import concourse.mybir as mybir
import concourse.tile as tile

B, C, W, K, F = 8, 256, 8192, 3, 256
COEF = 1.0 / float(np.sqrt(K * C))

P = 128
CT = C // P  # 2 contraction tiles
FT = F // P  # 2 output-partition tiles
WCHUNK = 2048  # feature chunk width
NJ = W // WCHUNK  # 4 chunks
WTILE = 512  # matmul moving-operand width (psum bank limit)
NI = WCHUNK // WTILE  # 4 w-tiles per chunk
XCOLS = WCHUNK + 2  # chunk + 1-col halo each side

N_WARM = 12  # dummy PE-warmup matmuls (N=256 each)

MAX_WAITS = 1  # walrus codegen in this container rejects >1 sync wait per inst


def _split_sync_waits(nc, limit=MAX_WAITS):
    """Move excess sem-waits onto NoOps inserted before the offending
    instruction (same engine, program order preserved)."""
    uid = 0
    for fn in nc.m.functions:
        for bb in fn.blocks:
            insts = bb.instructions
            changed = False
            newlist = []
            for ins in insts:
                si = ins.sync_info
                if si is not None and len(si.on_wait) > limit:
                    waits = list(si.on_wait)
                    keep = waits[-limit:]
                    excess = waits[:-limit]
                    for k in range(0, len(excess), limit):
                        nop = mybir.InstNoOp(name=f"waitsplit-{uid}", ins=[], outs=[])
                        uid += 1
                        nop.engine = ins.engine
                        nop.sync_info = mybir.SyncInfo(
                            on_wait=excess[k : k + limit], on_update=[]
                        )
                        newlist.append(nop)
                    ins.sync_info = mybir.SyncInfo(
                        on_wait=keep, on_update=list(si.on_update)
                    )
                    changed = True
                newlist.append(ins)
            if changed:
                bb.instructions = newlist


def _conv1dmod_body(tc, feat, style, kern, out):
    nc = tc.nc
    f32 = mybir.dt.float32
    bf16 = mybir.dt.bfloat16

    with (
        tc.tile_pool(name="xbuf", bufs=1) as xbuf,
        tc.tile_pool(name="xraw", bufs=2) as xraw_pool,
        tc.tile_pool(name="wbuf", bufs=1) as wbuf,
        tc.tile_pool(name="stage", bufs=3) as stage_pool,
        tc.tile_pool(name="psum", bufs=7, space="PSUM") as psum_pool,
        tc.tile_pool(name="dpsum", bufs=1, space="PSUM") as dpsum_pool,
    ):
        # ---- PE warmup: dense dummy matmuls while the first DMAs fly.
        # The HAM clock gate needs ~3.4us of sustained PE activity to open
        # to K=8/8; without this the first ~20 real matmuls run at 1.2 GHz.
        wz = wbuf.tile([P, 256], bf16, tag="warmz")
        nc.vector.memset(wz[:], 0.0)
        wps = dpsum_pool.tile([P, 256], f32, tag="dpsum")
        for _ in range(N_WARM):
            nc.tensor.matmul(wps[:], wz[:, :P], wz[:], start=True, stop=True)

        # ---- small weight DMAs: style on the gpsimd (SWDGE) queue, each
        # kern ct-half as one strided 3D DMA [P, K, F] on the scalar HWDGE
        # queue (ahead of the ct1 feature pieces riding the same ring).
        ssty = wbuf.tile([P, CT], f32, tag="ssty")
        with nc.allow_non_contiguous_dma(reason="256-elem style vector"):
            nc.gpsimd.dma_start(ssty[:], style.rearrange("(o p) -> p o", p=P))
        kview = kern.rearrange("k (h p) f -> p k h f", p=P)  # [128, K, CT, F]
        ksb = [
            wbuf.tile([P, K, F], f32, tag=f"ksb_{ct}", name=f"ksb_{ct}")
            for ct in range(CT)
        ]
        for ct in range(CT):
            nc.scalar.dma_start(ksb[ct][:], kview[:, :, ct, :])

        # ---- feature tiles (bf16). Chunk 0: fp32 pieces on the two HWDGE
        # queues + engine converts (lowest latency to first matmul).
        # Chunks 1+: SWDGE cast-DMA straight into the bf16 tiles.
        xt = [[None] * NJ for _ in range(CT)]

        def alloc_xt(ct, j):
            t = xbuf.tile([P, XCOLS], bf16, tag=f"x_{ct}_{j}", name=f"x_{ct}_{j}")
            xt[ct][j] = t
            lo = j * WCHUNK - 1
            hi = j * WCHUNK + WCHUNK + 1
            dst_lo = 0
            if lo < 0:
                nc.vector.memset(t[:, 0:1], 0.0)
                dst_lo = 1
                lo = 0
            if hi > W:
                nc.vector.memset(t[:, XCOLS - 1 : XCOLS], 0.0)
                hi = W
            return t, lo, hi, dst_lo

        def emit_load_chunk0(ct, npieces, dma_eng, cvt_eng):
            crow = slice(ct * P, (ct + 1) * P)
            t, lo, hi, dst_lo = alloc_xt(ct, 0)
            raw = xraw_pool.tile(
                [P, XCOLS], f32, tag=f"xraw_{ct}", name=f"xraw_{ct}"
            )
            bounds = np.linspace(lo, hi, npieces + 1).astype(int)
            for p0, p1 in zip(bounds[:-1], bounds[1:]):
                ncols = int(p1 - p0)
                off = dst_lo + int(p0 - lo)
                dma_eng.dma_start(raw[:, off : off + ncols], feat[crow, p0:p1])
                cvt_eng(t[:, off : off + ncols], raw[:, off : off + ncols])

        emit_load_chunk0(0, 4, nc.sync, nc.vector.tensor_copy)
        emit_load_chunk0(1, 4, nc.scalar, nc.scalar.copy)

        def emit_load(j):
            for ct in range(CT):
                crow = slice(ct * P, (ct + 1) * P)
                t, lo, hi, dst_lo = alloc_xt(ct, j)
                nc.gpsimd.dma_start(t[:, dst_lo : dst_lo + (hi - lo)], feat[crow, lo:hi])

        # chunk-1 cast-loads ride the gpsimd queue right behind style
        emit_load(1)

        # ---- modulate weights (bf16 out) ----
        s1 = wbuf.tile([P, CT], f32, tag="s1")
        nc.vector.tensor_scalar(
            s1[:], ssty[:], 1.0, COEF, mybir.AluOpType.add, mybir.AluOpType.mult
        )
        wm = []
        for ct in range(CT):
            wmt = wbuf.tile([P, K, F], bf16, tag=f"wm_{ct}", name=f"wm_{ct}")
            nc.vector.tensor_scalar_mul(wmt[:], ksb[ct][:], s1[:, ct : ct + 1])
            wm.append(wmt)

        def emit_mms(j, ft):
            """NI psum accumulation groups for (chunk j, ft), weight-major:
            each (ct,k) stationary load feeds all NI moving tiles."""
            pss = [
                psum_pool.tile([P, WTILE], f32, tag="psum", name=f"ps_{j}_{ft}_{i}")
                for i in range(NI)
            ]
            for ct in range(CT):
                for k in range(K):
                    first = ct == 0 and k == 0
                    last = ct == CT - 1 and k == K - 1
                    wslice = wm[ct][:, k, ft * P : (ft + 1) * P]
                    for i in range(NI):
                        nc.tensor.matmul(
                            pss[i][:],
                            wslice,
                            xt[ct][j][:, i * WTILE + k : i * WTILE + k + WTILE],
                            start=first,
                            stop=last,
                            skip_group_check=True,
                        )
            return pss

        def emit_copies(j, ft, pss):
            """Demodulating PSUM->SBUF bf16 copies + bf16 output stores on
            the scalar HWDGE queue (disjoint from the sync load queue)."""
            st = stage_pool.tile([P, WCHUNK], bf16, tag="stage")
            for i, ps in enumerate(pss):
                nc.vector.tensor_scalar_mul(
                    st[:, i * WTILE : (i + 1) * WTILE], ps[:], denom[:, ft : ft + 1]
                )
            out_rows = slice(ft * P, (ft + 1) * P)
            # finer stores on the last chunk shorten the end-of-kernel tail
            npieces = 4 if j == NJ - 1 else 2
            piece = WCHUNK // npieces
            for h in range(npieces):
                out_cols = slice(j * WCHUNK + h * piece, j * WCHUNK + (h + 1) * piece)
                nc.scalar.dma_start(
                    out[out_rows, out_cols], st[:, h * piece : (h + 1) * piece]
                )

        # chunk-0 loads + its first matmul block go ahead of everything else
        pss00 = emit_mms(0, 0)

        # ---- demodulation scale: denom[f] = rsqrt(sum_{k,c} wm^2) ----
        # Emitted after the first conv block so the tiny demod matmuls do
        # not sit at the head of the in-order PE queue waiting on the DVE
        # square/sum chain.
        ssq = []
        for ct in range(CT):
            sqt = wbuf.tile([P, K, F], f32, tag=f"sq_{ct}", name=f"sq_{ct}")
            nc.vector.tensor_mul(sqt[:], wm[ct][:], wm[ct][:])
            sst = wbuf.tile([P, F], f32, tag=f"ssq_{ct}", name=f"ssq_{ct}")
            nc.vector.tensor_add(sst[:], sqt[:, 0], sqt[:, 1])
            nc.vector.tensor_add(sst[:], sst[:], sqt[:, 2])
            ssq.append(sst)
        ones = wbuf.tile([P, 1], f32, tag="ones")
        nc.vector.memset(ones[:], 1.0)
        dp = dpsum_pool.tile([P, FT], f32, tag="dpsum")
        for ft in range(FT):
            for ct in range(CT):
                nc.tensor.matmul(
                    dp[:, ft : ft + 1],
                    ssq[ct][:, ft * P : (ft + 1) * P],
                    ones[:],
                    start=(ct == 0),
                    stop=(ct == CT - 1),
                )
        denom = wbuf.tile([P, FT], f32, tag="denom")
        nc.scalar.activation(denom[:], dp[:], mybir.ActivationFunctionType.Sqrt)
        nc.vector.reciprocal(denom[:], denom[:])

        # ---- conv: chunk loads stay one chunk ahead of the matmul stream ----
        emit_copies(0, 0, pss00)
        emit_copies(0, 1, emit_mms(0, 1))
        for j in range(1, NJ):
            if j + 1 < NJ:
                emit_load(j + 1)
            for ft in range(FT):
                emit_copies(j, ft, emit_mms(j, ft))


def build_bass():
    nc = bass.Bass(name="conv1dmod")
    feat = nc.dram_tensor("feature", [C, W], mybir.dt.float32, kind="ExternalInput")
    style = nc.dram_tensor("style", [C], mybir.dt.float32, kind="ExternalInput")
    kern = nc.dram_tensor("kern", [K, C, F], mybir.dt.float32, kind="ExternalInput")
    out = nc.dram_tensor("out", [F, W], mybir.dt.bfloat16, kind="ExternalOutput")
    with tile.TileContext(nc) as tc:
        _conv1dmod_body(tc, feat, style, kern, out)
    _split_sync_waits(nc)
    return nc


_NC_CACHE = None


def kernel(feature, style, kernel):
    """Full-input entry point: shard over batch across 8 cores, run, gather."""
    global _NC_CACHE
    from concourse.bass_utils import run_bass_kernel_spmd

    if _NC_CACHE is None:
        _NC_CACHE = build_bass()
    nc = _NC_CACHE

    feature = np.ascontiguousarray(feature, dtype=np.float32)
    style = np.ascontiguousarray(style, dtype=np.float32)
    kernel = np.ascontiguousarray(kernel, dtype=np.float32)

    in_maps = [
        {"feature": feature[b], "style": style[b], "kern": kernel} for b in range(B)
    ]
    res = run_bass_kernel_spmd(nc, in_maps, core_ids=list(range(B)))
    return np.stack(
        [np.asarray(r["out"]).astype(np.float32) for r in res.results], axis=0
    )


# revision 5
# speedup vs baseline: 1.1414x; 1.1414x over previous
"""Trainium2 Bass kernel for modulated conv1d (StyleGAN-style Conv1DMod).

Reference computation (per batch sample b):
  wm[k,c,f]  = kern[k,c,f] * coef * (style[b,c] + 1)        (modulate)
  denom[f]   = rsqrt(sum_{k,c} wm[k,c,f]^2)                 (demodulate)
  out[b,f,w] = denom[f] * sum_{k,c} wm[k,c,f] * feat[b,c,w+k-1]   (SAME conv)

Sharding: data-parallel over batch B=8 -> one sample per NeuronCore.
Demodulation is a per-(b,f) linear scale, so it is applied to the conv
*output* tiles (whose partition dim is f) instead of rescaling weights.

v4 structure (v1 trace: first MM at 14.6us, PE cold till 23.7us; v3
showed SWDGE cast-DMA throttles every DMA queue -> back to HWDGE):
  - the conv runs in bf16 (weights and features; fp32 PSUM accumulate):
    same 1 col/cycle PE rate as fp32r, no "producer must round"
    verifier constraint, FWL weight loads
  - features are DMA'd fp32: ct0 on the sync HWDGE ring, ct1 (+style,
    kern) on the scalar HWDGE ring; fp32->bf16 converts split across
    the vector (ct0) and scalar (ct1) engines; chunk 0 lands in 4
    pieces per half so the first matmul group starts ~10us in
  - a block of dummy bf16 matmuls at the head keeps the PE busy during
    the initial DMA wait so the HAM clock-gate opens (K=8/8) before the
    real matmul stream starts
  - weight-major matmul order: one (ct,k) weight load feeds the 4
    accumulating matmuls of a chunk (LDWEIGHTS 4x amortized)
  - conv output is demodulated into bf16 staging tiles and stored as
    bf16 on the scalar ring (halves store traffic + kernel tail);
    host upcasts to fp32
"""

import numpy as np

import concourse.bass as bass
import concourse.mybir as mybir
import concourse.tile as tile

B, C, W, K, F = 8, 256, 8192, 3, 256
COEF = 1.0 / float(np.sqrt(K * C))

P = 128
CT = C // P  # 2 contraction tiles
FT = F // P  # 2 output-partition tiles
WCHUNK = 2048  # feature chunk width
NJ = W // WCHUNK  # 4 chunks
WTILE = 512  # matmul moving-operand width (psum bank limit)
NI = WCHUNK // WTILE  # 4 w-tiles per chunk
XCOLS = WCHUNK + 2  # chunk + 1-col halo each side

N_WARM = 12  # dummy PE-warmup matmuls (N=256 each)

MAX_WAITS = 1  # walrus codegen in this container rejects >1 sync wait per inst


def _split_sync_waits(nc, limit=MAX_WAITS):
    """Move excess sem-waits onto NoOps inserted before the offending
    instruction (same engine, program order preserved)."""
    uid = 0
    for fn in nc.m.functions:
        for bb in fn.blocks:
            insts = bb.instructions
            changed = False
            newlist = []
            for ins in insts:
                si = ins.sync_info
                if si is not None and len(si.on_wait) > limit:
                    waits = list(si.on_wait)
                    keep = waits[-limit:]
                    excess = waits[:-limit]
                    for k in range(0, len(excess), limit):
                        nop = mybir.InstNoOp(name=f"waitsplit-{uid}", ins=[], outs=[])
                        uid += 1
                        nop.engine = ins.engine
                        nop.sync_info = mybir.SyncInfo(
                            on_wait=excess[k : k + limit], on_update=[]
                        )
                        newlist.append(nop)
                    ins.sync_info = mybir.SyncInfo(
                        on_wait=keep, on_update=list(si.on_update)
                    )
                    changed = True
                newlist.append(ins)
            if changed:
                bb.instructions = newlist


def _conv1dmod_body(tc, feat, style, kern, out):
    nc = tc.nc
    f32 = mybir.dt.float32
    bf16 = mybir.dt.bfloat16

    with (
        tc.tile_pool(name="xbuf", bufs=1) as xbuf,
        tc.tile_pool(name="xraw", bufs=2) as xraw_pool,
        tc.tile_pool(name="wbuf", bufs=1) as wbuf,
        tc.tile_pool(name="stage", bufs=3) as stage_pool,
        tc.tile_pool(name="psum", bufs=7, space="PSUM") as psum_pool,
        tc.tile_pool(name="dpsum", bufs=1, space="PSUM") as dpsum_pool,
    ):
        # ---- PE warmup: dense dummy matmuls while the first DMAs fly.
        # The HAM clock gate needs ~3.4us of sustained PE activity to open
        # to K=8/8; without this the first ~20 real matmuls run at 1.2 GHz.
        wz = wbuf.tile([P, 256], bf16, tag="warmz")
        nc.vector.memset(wz[:], 0.0)
        wps = dpsum_pool.tile([P, 256], f32, tag="dpsum")
        for _ in range(N_WARM):
            nc.tensor.matmul(wps[:], wz[:, :P], wz[:], start=True, stop=True)

        # ---- small weight DMAs, all on the scalar HWDGE ring ahead of the
        # ct1 feature pieces: style scatter first (tiny), then each kern
        # ct-half as one strided 3D DMA [P, K, F].
        ssty = wbuf.tile([P, CT], f32, tag="ssty")
        with nc.allow_non_contiguous_dma(reason="256-elem style vector"):
            nc.scalar.dma_start(ssty[:], style.rearrange("(o p) -> p o", p=P))
        kview = kern.rearrange("k (h p) f -> p k h f", p=P)  # [128, K, CT, F]
        ksb = [
            wbuf.tile([P, K, F], f32, tag=f"ksb_{ct}", name=f"ksb_{ct}")
            for ct in range(CT)
        ]
        for ct in range(CT):
            nc.scalar.dma_start(ksb[ct][:], kview[:, :, ct, :])

        # ---- feature tiles: fp32 DMA (ct0 -> sync ring, ct1 -> scalar
        # ring) + engine convert to bf16 (ct0 -> vector, ct1 -> scalar).
        xt = [[None] * NJ for _ in range(CT)]
        dma_eng = [nc.sync, nc.scalar]
        cvt_eng = [nc.vector.tensor_copy, nc.scalar.copy]

        def emit_load(j, npieces=1):
            for ct in range(CT):
                crow = slice(ct * P, (ct + 1) * P)
                t = xbuf.tile([P, XCOLS], bf16, tag=f"x_{ct}_{j}", name=f"x_{ct}_{j}")
                xt[ct][j] = t
                raw = xraw_pool.tile(
                    [P, XCOLS], f32, tag=f"xraw_{ct}", name=f"xraw_{ct}_{j}"
                )
                lo = j * WCHUNK - 1
                hi = j * WCHUNK + WCHUNK + 1
                dst_lo = 0
                if lo < 0:
                    nc.vector.memset(t[:, 0:1], 0.0)
                    dst_lo = 1
                    lo = 0
                if hi > W:
                    nc.vector.memset(t[:, XCOLS - 1 : XCOLS], 0.0)
                    hi = W
                bounds = np.linspace(lo, hi, npieces + 1).astype(int)
                for p0, p1 in zip(bounds[:-1], bounds[1:]):
                    ncols = int(p1 - p0)
                    off = dst_lo + int(p0 - lo)
                    dma_eng[ct].dma_start(raw[:, off : off + ncols], feat[crow, p0:p1])
                    cvt_eng[ct](t[:, off : off + ncols], raw[:, off : off + ncols])

        emit_load(0, npieces=4)

        # ---- modulate weights (bf16 out) ----
        s1 = wbuf.tile([P, CT], f32, tag="s1")
        nc.vector.tensor_scalar(
            s1[:], ssty[:], 1.0, COEF, mybir.AluOpType.add, mybir.AluOpType.mult
        )
        wm = []
        for ct in range(CT):
            wmt = wbuf.tile([P, K, F], bf16, tag=f"wm_{ct}", name=f"wm_{ct}")
            nc.vector.tensor_scalar_mul(wmt[:], ksb[ct][:], s1[:, ct : ct + 1])
            wm.append(wmt)

        def emit_mms(j, ft):
            """NI psum accumulation groups for (chunk j, ft), weight-major:
            each (ct,k) stationary load feeds all NI moving tiles."""
            pss = [
                psum_pool.tile([P, WTILE], f32, tag="psum", name=f"ps_{j}_{ft}_{i}")
                for i in range(NI)
            ]
            for ct in range(CT):
                for k in range(K):
                    first = ct == 0 and k == 0
                    last = ct == CT - 1 and k == K - 1
                    wslice = wm[ct][:, k, ft * P : (ft + 1) * P]
                    for i in range(NI):
                        nc.tensor.matmul(
                            pss[i][:],
                            wslice,
                            xt[ct][j][:, i * WTILE + k : i * WTILE + k + WTILE],
                            start=first,
                            stop=last,
                            skip_group_check=True,
                        )
            return pss

        def emit_copies(j, ft, pss):
            """Demodulating PSUM->SBUF bf16 copies + bf16 output stores on
            the scalar HWDGE ring (disjoint from the sync load ring)."""
            st = stage_pool.tile([P, WCHUNK], bf16, tag="stage")
            for i, ps in enumerate(pss):
                nc.vector.tensor_scalar_mul(
                    st[:, i * WTILE : (i + 1) * WTILE], ps[:], denom[:, ft : ft + 1]
                )
            out_rows = slice(ft * P, (ft + 1) * P)
            # finer stores on the last chunk shorten the end-of-kernel tail
            npieces = 4 if j == NJ - 1 else 2
            piece = WCHUNK // npieces
            for h in range(npieces):
                out_cols = slice(j * WCHUNK + h * piece, j * WCHUNK + (h + 1) * piece)
                nc.scalar.dma_start(
                    out[out_rows, out_cols], st[:, h * piece : (h + 1) * piece]
                )

        # chunk-0 loads + its first matmul block go ahead of everything else
        pss00 = emit_mms(0, 0)

        # ---- demodulation scale: denom[f] = rsqrt(sum_{k,c} wm^2) ----
        # Emitted after the first conv block so the tiny demod matmuls do
        # not sit at the head of the in-order PE queue waiting on the DVE
        # square/sum chain.
        ssq = []
        for ct in range(CT):
            sqt = wbuf.tile([P, K, F], f32, tag=f"sq_{ct}", name=f"sq_{ct}")
            nc.vector.tensor_mul(sqt[:], wm[ct][:], wm[ct][:])
            sst = wbuf.tile([P, F], f32, tag=f"ssq_{ct}", name=f"ssq_{ct}")
            nc.vector.tensor_add(sst[:], sqt[:, 0], sqt[:, 1])
            nc.vector.tensor_add(sst[:], sst[:], sqt[:, 2])
            ssq.append(sst)
        ones = wbuf.tile([P, 1], f32, tag="ones")
        nc.vector.memset(ones[:], 1.0)
        dp = dpsum_pool.tile([P, FT], f32, tag="dpsum")
        for ft in range(FT):
            for ct in range(CT):
                nc.tensor.matmul(
                    dp[:, ft : ft + 1],
                    ssq[ct][:, ft * P : (ft + 1) * P],
                    ones[:],
                    start=(ct == 0),
                    stop=(ct == CT - 1),
                )
        denom = wbuf.tile([P, FT], f32, tag="denom")
        nc.scalar.activation(denom[:], dp[:], mybir.ActivationFunctionType.Sqrt)
        nc.vector.reciprocal(denom[:], denom[:])

        # ---- conv: chunk loads stay one chunk ahead of the matmul stream ----
        emit_load(1)
        emit_copies(0, 0, pss00)
        emit_copies(0, 1, emit_mms(0, 1))
        for j in range(1, NJ):
            if j + 1 < NJ:
                emit_load(j + 1)
            for ft in range(FT):
                emit_copies(j, ft, emit_mms(j, ft))


def build_bass():
    nc = bass.Bass(name="conv1dmod")
    feat = nc.dram_tensor("feature", [C, W], mybir.dt.float32, kind="ExternalInput")
    style = nc.dram_tensor("style", [C], mybir.dt.float32, kind="ExternalInput")
    kern = nc.dram_tensor("kern", [K, C, F], mybir.dt.float32, kind="ExternalInput")
    out = nc.dram_tensor("out", [F, W], mybir.dt.bfloat16, kind="ExternalOutput")
    with tile.TileContext(nc) as tc:
        _conv1dmod_body(tc, feat, style, kern, out)
    _split_sync_waits(nc)
    return nc


_NC_CACHE = None


def kernel(feature, style, kernel):
    """Full-input entry point: shard over batch across 8 cores, run, gather."""
    global _NC_CACHE
    from concourse.bass_utils import run_bass_kernel_spmd

    if _NC_CACHE is None:
        _NC_CACHE = build_bass()
    nc = _NC_CACHE

    feature = np.ascontiguousarray(feature, dtype=np.float32)
    style = np.ascontiguousarray(style, dtype=np.float32)
    kernel = np.ascontiguousarray(kernel, dtype=np.float32)

    in_maps = [
        {"feature": feature[b], "style": style[b], "kern": kernel} for b in range(B)
    ]
    res = run_bass_kernel_spmd(nc, in_maps, core_ids=list(range(B)))
    return np.stack(
        [np.asarray(r["out"]).astype(np.float32) for r in res.results], axis=0
    )
